# revision 10
# baseline (speedup 1.0000x reference)
"""Trainium2 Bass kernel for nn_MetaRouter (dense_transformer).

Contract: kernel(**inputs) takes FULL unsharded inputs (as produced by
reference.setup_inputs()) and returns the FULL [B, D] logits, matching
reference.reference(**inputs).

Strategy:
  - Data-parallel over batch: B=16 split as 2 rows per core x 8 cores.
    All parameters replicated. No collectives.
  - Host side: tokens with attention_mask==0 get softmax weight exactly 0
    for every query, so BOTH of a core's rows are compacted into ONE
    contiguous token stream (row0 tokens, then row1 tokens), padded to a
    multiple of 128 ("joint packing": ~17 tiles instead of 2x9).  Tiles
    that straddle the row boundary get TWO exp-weight vectors (one per
    row view, via per-view -1e9 pad biases), so each row's context
    accumulates only its own tokens.  ts is pre-cast to bf16 and
    pre-transposed into [tile, 128 feat, tok] chunks so the chip never
    transposes it.  The 17 attention queries are folded into the
    projection weight matrix as extra columns:
        Q_hat = W @ q - w_bar * colsum(q)   (w_bar = row-mean of W)
    which makes raw_score[s,q] = ts_s . Q_hat[:,q] = v_s.q - mu_s*sum(q),
    i.e. the LN mean-correction is pre-applied; only the per-token rstd
    scaling remains.  So scores cost 17 extra matmul columns, not a
    separate pass, and x^T never needs to exist on chip.
  - Softmax denominators and the LN mean shift both cancel inside the
    downstream LayerNorms (LN is invariant to positive scaling and
    uniform shifts), so the context sums use unnormalized exp weights
    against the raw (pre-LN) projections, with the per-token rstd folded
    into the exp weights.  No reduce_max, no reciprocal, no renorm.
  - Per tile of 128 tokens: 64 matmuls (32 k-chunks x 2 PSUM splits of
    256+273 columns; one PSUM bank each, LDWEIGHTS fully hidden), then
    DVE does bn_stats/bn_aggr + a bitcast-Newton rsqrt (keeps the ACT
    table pinned on Exp), ACT does exp(rstd*raw + padbias) in a single
    fused instruction, and the row contexts accumulate incrementally in
    one PSUM bank (row0 at partitions 0:17, row1 at 32:49 via the PE
    column-group mechanism) interleaved into the projection stream.
  - Startup: the first ~8us (engine boot + DMA bring-up) are covered by
    garbage warm-up matmuls that also pre-warm the PE HAM clock gate;
    tiles 0 and 1 are then co-processed CHUNK-MAJOR (tile1 lagging six
    k-chunks) so each arriving W' chunk feeds two tiles' matmuls --
    this halves the W'-bandwidth-per-flop while W' (4.3MB) streams in.
    W' and the first two ts tiles are striped across all three DMA
    queues (2 HWDGE + SWDGE) in consumption order.
  - Tail: both rows' context LN + fuse + FFN run ONCE batched (M=32
    instead of 2xM=16): stats via bn_stats on the two partition groups,
    bitcast-Newton rsqrt, PE transposes (row groups 0/32 run
    concurrently), gelu-FFN with all weights pre-chunked, output head
    folded with the temperature on the host.  ACT function tables (Exp,
    Gelu) are preloaded off the critical path by dummy activations.
"""

import os

import numpy as np
import ml_dtypes

import concourse.bass as bass
import concourse.bacc as bacc
import concourse.tile as tile
from concourse import mybir

P = 128
H = 512
TOKD = 4096
KC = TOKD // P    # 32 k-chunks of the projection contraction
NQ = 17           # 1 global + 16 domain queries
WTOT = H + NQ     # 529 projection output columns
SPL = 256         # W column split; region B holds W[256:] + scores
D = 16
B = 16
S = 2048
N_CORES = 8
B_LOCAL = B // N_CORES
EPS = 1e-5
F32 = mybir.dt.float32
I32 = mybir.dt.int32
BF16 = mybir.dt.bfloat16
MAGIC = 0x5F3759DF
LAG = 6           # tile1 k-chunk lag during the startup co-processing
NWARM = 22        # garbage warm-up matmuls covering engine/DMA boot


def build_nc(TT: int, T1S: int, T0E: int, b_out_s: float, skip=frozenset()):
    """Per-core Bass program: TT jointly-packed tiles, row0 ctx over tiles
    [0,T0E), row1 over [T1S,TT); tiles in [T1S,T0E) carry two exp views."""
    NOV = T0E - T1S

    nc = bacc.Bacc("TRN2", target_bir_lowering=False, num_swdge_queues=2)

    ts = nc.declare_dram_parameter("ts", [TT * P, TOKD], BF16, isOutput=False)
    pb = nc.declare_dram_parameter("pb", [P, TT + NOV], F32, isOutput=False)
    wp = nc.declare_dram_parameter("wp", [P, KC * WTOT], BF16, isOutput=False)
    bprow = nc.declare_dram_parameter("bprow", [1, WTOT], BF16, isOutput=False)
    tg = nc.declare_dram_parameter("tg", [1, H], F32, isOutput=False)
    tb = nc.declare_dram_parameter("tb", [1, H], F32, isOutput=False)
    cg = nc.declare_dram_parameter("cg", [NQ, H], F32, isOutput=False)
    cb = nc.declare_dram_parameter("cb", [NQ, H], F32, isOutput=False)
    fg = nc.declare_dram_parameter("fg", [1, H], F32, isOutput=False)
    fb = nc.declare_dram_parameter("fb", [1, H], F32, isOutput=False)
    w1 = nc.declare_dram_parameter("w1", [P, 8 * H], BF16, isOutput=False)
    bf1 = nc.declare_dram_parameter("bf1", [1, H], BF16, isOutput=False)
    w2 = nc.declare_dram_parameter("w2", [P, 4 * H], BF16, isOutput=False)
    bf2 = nc.declare_dram_parameter("bf2", [1, H], BF16, isOutput=False)
    wo = nc.declare_dram_parameter("wo", [1, H], F32, isOutput=False)
    idm = nc.declare_dram_parameter("idm", [2 * D, 2 * D + NQ], BF16,
                                    isOutput=False)
    out = nc.declare_dram_parameter("out", [B_LOCAL * D, 1], F32, isOutput=True)

    with tile.TileContext(nc) as tc:
        _emit(tc, nc, TT, T1S, T0E, b_out_s, skip,
              ts=ts, pb=pb, wp=wp, bprow=bprow, tg=tg, tb=tb, cg=cg, cb=cb,
              fg=fg, fb=fb, w1=w1, bf1=bf1, w2=w2, bf2=bf2, wo=wo, idm=idm,
              out=out)
    nc.compile()
    return nc


def _emit(tc, nc, TT, T1S, T0E, b_out_s, skip, *, ts, pb, wp, bprow, tg, tb,
          cg, cb, fg, fb, w1, bf1, w2, bf2, wo, idm, out):
    from contextlib import ExitStack
    NOV = T0E - T1S
    M2 = 2 * D        # both rows' FFN batched: 32 output rows
    R1 = 32           # row1 ctx partition base (PE column-group aligned)
    ctx = ExitStack()
    with ctx:
        const = ctx.enter_context(tc.tile_pool(name="const", bufs=1))
        tsp = ctx.enter_context(tc.tile_pool(name="tsp", bufs=6))
        xp = ctx.enter_context(tc.tile_pool(name="xp", bufs=1))
        lnp = ctx.enter_context(tc.tile_pool(name="lnp", bufs=2))
        p2 = ctx.enter_context(tc.tile_pool(name="p2", bufs=2))
        psx = ctx.enter_context(tc.tile_pool(name="psx", bufs=2, space="PSUM"))
        ctxp = ctx.enter_context(tc.tile_pool(name="ctxp", bufs=1, space="PSUM"))
        pst = ctx.enter_context(tc.tile_pool(name="pst", bufs=1, space="PSUM"))
        ffnp = ctx.enter_context(tc.tile_pool(name="ffnp", bufs=1, space="PSUM"))

        # ---- SBUF-resident state ----
        x_sb = xp.tile([P, TT, H], BF16)
        pexpT = xp.tile([P, TT + NOV, NQ], BF16)
        logit_sb = xp.tile([M2, 1], F32)

        # ---- cheap constants + PE warm-up (no DMA dependencies) ----
        ones_row = const.tile([1, P], BF16)
        nc.vector.memset(ones_row, 1.0)
        ones_col = const.tile([P, D], BF16)
        nc.vector.memset(ones_col, 1.0)
        magic = const.tile([P, 1], I32)
        nc.vector.memset(magic, MAGIC)
        # warm-up operand must be initialized (race detector) and cheap
        nc.vector.memset(x_sb[:, 0, :], 0.0)

        # garbage matmuls: occupy the PE through engine/DMA bring-up so the
        # HAM clock-gate is at 8/8 before real work, and the PE never sees
        # a >3.4us idle window at the start.
        wtile = psx.tile([P, 2 * H], F32, tag="px", name="warm")
        for _ in range(NWARM):
            nc.tensor.matmul(wtile[:, 0:SPL], lhsT=x_sb[:, 0, 0:P],
                             rhs=x_sb[:, 0, 0:SPL], start=True, stop=True,
                             skip_group_check=True)

        # identities come from the host (idm): [0:32,0:32]=I32 for the FFN
        # transposes; [0:17,32:49] and [32:49,32:49]=I17 for the two ctx
        # transpose row-groups.
        idm_sb = const.tile([M2, M2 + NQ], BF16)

        # ---- startup DMA stream, striped across the three queues in
        # consumption order (phase A consumes W chunk-major over 2 tiles).
        w_sb = const.tile([P, KC, WTOT], BF16)
        _wp = wp.ap().rearrange("p (c w) -> p c w", w=WTOT)
        pb_sb = const.tile([P, TT + NOV], F32)
        nc.sync.dma_start(out=pb_sb, in_=pb.ap())

        prefetched = {}

        def dma_tile(t, eng=None):
            if eng is None:
                eng = (nc.gpsimd, nc.sync, nc.scalar)[t % 3]
            tt = tsp.tile([P, KC * P], BF16, tag="ts")
            eng.dma_start(out=tt, in_=ts.ap()[t * P:(t + 1) * P, :])
            prefetched[t] = tt

        def dma_tile_blocks(t, eng_list):
            """4 block-DMAs of 8 k-chunks each (2KB lines) for one tile."""
            tt = tsp.tile([P, KC * P], BF16, tag="ts")
            tv = tt.rearrange("p (c s) -> p c s", s=P)
            sv = ts.ap()[t * P:(t + 1) * P, :].rearrange("p (c s) -> p c s", s=P)
            for b, eng in enumerate(eng_list):
                eng.dma_start(out=tv[:, 8 * b:8 * b + 8, :],
                              in_=sv[:, 8 * b:8 * b + 8, :])
            prefetched[t] = tt

        def dma_w(q, eng):
            """One 4-chunk block of W' (~0.54MB, 8.5KB lines)."""
            eng.dma_start(out=w_sb[:, 4 * q:4 * q + 4, :],
                          in_=_wp[:, 4 * q:4 * q + 4, :])

        # need order: Wq0,ts0b0 | Wq1 | ts1b0 | Wq2,ts0b1 | Wq3 | ts1b1 |
        #             Wq4,ts0b2 | Wq5 | ts1b2 | Wq6,ts0b3 | Wq7 | ts1b3
        # sync/scalar (HWDGE) lead; gpsimd (SWDGE) boots ~3us later so it
        # carries items needed from mid-phase-A on.
        dma_w(0, nc.sync)
        ts0 = tsp.tile([P, KC * P], BF16, tag="ts", name="ts0")
        t0v = ts0.rearrange("p (c s) -> p c s", s=P)
        s0v = ts.ap()[0:P, :].rearrange("p (c s) -> p c s", s=P)
        ts1 = tsp.tile([P, KC * P], BF16, tag="ts", name="ts1")
        t1v = ts1.rearrange("p (c s) -> p c s", s=P)
        s1v = ts.ap()[P:2 * P, :].rearrange("p (c s) -> p c s", s=P)
        prefetched[0] = ts0
        prefetched[1] = ts1

        nc.scalar.dma_start(out=t0v[:, 0:8, :], in_=s0v[:, 0:8, :])   # ts0.b0
        dma_w(1, nc.scalar)
        nc.sync.dma_start(out=t1v[:, 0:8, :], in_=s1v[:, 0:8, :])     # ts1.b0
        dma_w(2, nc.scalar)
        nc.gpsimd.dma_start(out=t0v[:, 8:16, :], in_=s0v[:, 8:16, :])  # ts0.b1
        dma_w(3, nc.sync)
        nc.scalar.dma_start(out=t1v[:, 8:16, :], in_=s1v[:, 8:16, :])  # ts1.b1
        dma_w(4, nc.gpsimd)
        nc.sync.dma_start(out=t0v[:, 16:24, :], in_=s0v[:, 16:24, :])  # ts0.b2
        dma_w(5, nc.scalar)
        nc.gpsimd.dma_start(out=t1v[:, 16:24, :], in_=s1v[:, 16:24, :])  # ts1.b2
        dma_w(6, nc.sync)
        nc.scalar.dma_start(out=t0v[:, 24:32, :], in_=s0v[:, 24:32, :])  # ts0.b3
        dma_w(7, nc.gpsimd)
        nc.sync.dma_start(out=t1v[:, 24:32, :], in_=s1v[:, 24:32, :])  # ts1.b3
        # next tiles land behind the startup stream, block-striped so tile 2/3
        # compute can start on partial arrivals (the stream is DMA-paced
        # until ~tile 3)
        dma_tile_blocks(2, (nc.sync, nc.scalar, nc.gpsimd, nc.sync))
        dma_tile_blocks(3, (nc.scalar, nc.gpsimd, nc.sync, nc.scalar))
        dma_tile(4, nc.gpsimd)
        dma_tile(5, nc.scalar)

        # ---- ACT table preload: first Exp use must not pay the table load
        dummy_act = lnp.tile([1, 4], F32, tag="dummy_act")
        nc.scalar.activation(out=dummy_act, in_=ones_row[:, 0:4],
                             func=mybir.ActivationFunctionType.Exp)

        # ---- tail constants (DMA-idle region; needed only at the tail) ----
        def bcast(dram, parts, dt=F32):
            t = const.tile([parts, H], dt, tag=f"c_{dram.name}")
            a = dram.ap()
            nc.sync.dma_start(
                out=t, in_=bass.AP(tensor=a.tensor, offset=a.offset,
                                   ap=[[0, parts]] + list(a.ap[1:])))
            return t

        w1_sb = const.tile([P, 8, H], BF16)
        nc.sync.dma_start(out=w1_sb, in_=w1.ap().rearrange("p (c h) -> p c h", h=H))
        w2_sb = const.tile([P, 4, H], BF16)
        nc.scalar.dma_start(out=w2_sb, in_=w2.ap().rearrange("p (c h) -> p c h", h=H))
        wo_sb = bcast(wo, M2)
        if "tln" not in skip:
            tg_sb = bcast(tg, P)
            tb_sb = bcast(tb, P)
        if "gcln" not in skip:
            cg_sb = const.tile([NQ, H], F32)
            nc.scalar.dma_start(out=cg_sb, in_=cg.ap())
            cb_sb = const.tile([NQ, H], F32)
            nc.scalar.dma_start(out=cb_sb, in_=cb.ap())
        if "fln" not in skip:
            fg_sb = bcast(fg, M2)
            fb_sb = bcast(fb, M2)
        if "bf1" not in skip:
            bf1_sb = const.tile([1, H], BF16)
            nc.sync.dma_start(out=bf1_sb, in_=bf1.ap())
        if "bf2" not in skip:
            bf2_sb = const.tile([1, H], BF16)
            nc.sync.dma_start(out=bf2_sb, in_=bf2.ap())
        if "bp" not in skip:
            bprow_sb = const.tile([1, WTOT], BF16)
            nc.sync.dma_start(out=bprow_sb, in_=bprow.ap())
        nc.sync.dma_start(out=idm_sb, in_=idm.ap())

        def rsqrt(ve, parts, tag, iters=1):
            """y ~= (ve)^-0.5 via bitcast seed + Newton steps (DVE only)."""
            y = lnp.tile([parts, 1], F32, tag=f"y_{tag}")
            sh = lnp.tile([parts, 1], I32, tag=f"sh_{tag}")
            nc.vector.tensor_scalar(out=sh, in0=ve.bitcast(I32), scalar1=1,
                                    scalar2=None,
                                    op0=mybir.AluOpType.arith_shift_right)
            nc.vector.tensor_tensor(out=y.bitcast(I32), in0=magic[:parts],
                                    in1=sh, op=mybir.AluOpType.subtract)
            t1 = lnp.tile([parts, 1], F32, tag=f"t1_{tag}")
            hh = lnp.tile([parts, 1], F32, tag=f"h_{tag}")
            for _ in range(iters):
                nc.vector.tensor_mul(out=t1, in0=y, in1=y)
                nc.vector.tensor_mul(out=t1, in0=t1, in1=ve)
                nc.vector.tensor_scalar(out=hh, in0=t1, scalar1=-0.5,
                                        scalar2=1.5, op0=mybir.AluOpType.mult,
                                        op1=mybir.AluOpType.add)
                nc.vector.tensor_mul(out=y, in0=y, in1=hh)
            return y

        psums = {}

        def proj_alloc(t):
            tsT = prefetched.pop(t).rearrange("p (c s) -> p c s", s=P)
            px = psx.tile([P, 2 * H], F32, tag="px", name="px")
            psums[t] = px
            return tsT, px

        def proj_chunk(tsT, px, k):
            first, last = k == 0, k == KC - 1 and "bp" in skip
            nc.tensor.matmul(px[:, 0:SPL], lhsT=tsT[:, k, :],
                             rhs=w_sb[:, k, 0:SPL], start=first, stop=last,
                             skip_group_check=True)
            nc.tensor.matmul(px[:, H:H + WTOT - SPL], lhsT=tsT[:, k, :],
                             rhs=w_sb[:, k, SPL:], start=first, stop=last,
                             skip_group_check=True)

        def proj_bias(px):
            if "bp" not in skip:
                nc.tensor.matmul(px[:, 0:SPL], lhsT=ones_row,
                                 rhs=bprow_sb[:, 0:SPL], start=False,
                                 stop=True, skip_group_check=True)
                nc.tensor.matmul(px[:, H:H + WTOT - SPL], lhsT=ones_row,
                                 rhs=bprow_sb[:, SPL:], start=False,
                                 stop=True, skip_group_check=True)

        def proj(t):
            tsT, px = proj_alloc(t)
            for k in range(KC):
                proj_chunk(tsT, px, k)
            proj_bias(px)

        def exp_slots(t):
            """pexpT/pb column(s) for tile t: primary view + overlap view."""
            slots = [t]
            if T1S <= t < T0E:
                slots.append(TT + (t - T1S))
            return slots

        def post(t):
            """Stats + x store + exp-weights for tile t (DVE/ACT work)."""
            px = psums.pop(t)
            vreg = px.rearrange("p (r x) -> p r x", x=H)[:, :, 0:SPL]
            stats = lnp.tile([P, 12], F32, tag="stats")
            nc.vector.bn_stats(out=stats[:, 0:6], in_=vreg[:, 0, :])
            nc.vector.bn_stats(out=stats[:, 6:12], in_=vreg[:, 1, :])
            mv = lnp.tile([P, 2], F32, tag="mv")
            nc.vector.bn_aggr(out=mv, in_=stats)
            ve = lnp.tile([P, 1], F32, tag="ve")
            nc.vector.tensor_scalar_add(out=ve, in0=mv[:, 1:2], scalar1=EPS)
            rstd = rsqrt(ve, P, "p1")
            # attn weights first: they gate the trailing ctx matmul
            for sl in exp_slots(t):
                nc.scalar.activation(out=pexpT[:, sl, :],
                                     in_=px[:, H + H - SPL:H + WTOT - SPL],
                                     func=mybir.ActivationFunctionType.Exp,
                                     bias=pb_sb[:, sl:sl + 1], scale=rstd)
            xv = x_sb[:, t, :].rearrange("p (r x) -> p r x", x=SPL)
            if "tln" in skip:
                for sl in exp_slots(t):
                    nc.vector.tensor_scalar_mul(out=pexpT[:, sl, :],
                                                in0=pexpT[:, sl, :],
                                                scalar1=rstd)
                # store raw v; rstd folds into the attn weights, mu cancels
                nc.vector.tensor_copy(out=xv, in_=vreg)
            else:
                xa = lnp.tile([P, H], F32, tag="xa")
                nc.vector.tensor_scalar(
                    out=xa.rearrange("p (r x) -> p r x", x=SPL), in0=vreg,
                    scalar1=mv[:, 0:1], scalar2=rstd,
                    op0=mybir.AluOpType.subtract, op1=mybir.AluOpType.mult)
                xg = lnp.tile([P, H], F32, tag="xg")
                nc.vector.tensor_mul(out=xg, in0=xa, in1=tg_sb)
                nc.vector.tensor_add(out=x_sb[:, t, :], in0=xg, in1=tb_sb)

        ctx_ps_box = {}

        def ctx_mm(u):
            """Accumulate row contexts for tile u (one PSUM bank per row,
            both at partition base 0)."""
            if u == 0:
                ctx_ps_box[0] = ctxp.tile([NQ, H], F32, tag="ctx0", name="ctx0")
                ctx_ps_box[1] = ctxp.tile([NQ, H], F32, tag="ctx1", name="ctx1")
            if u < T0E:
                nc.tensor.matmul(ctx_ps_box[0], lhsT=pexpT[:, u, :],
                                 rhs=x_sb[:, u, :], start=(u == 0),
                                 stop=(u == T0E - 1), skip_group_check=True)
            if u >= T1S:
                sl = TT + (u - T1S) if u < T0E else u
                nc.tensor.matmul(ctx_ps_box[1], lhsT=pexpT[:, sl, :],
                                 rhs=x_sb[:, u, :], start=(u == T1S),
                                 stop=(u == TT - 1), skip_group_check=True)

        def tail():
            """Both rows' ctx LN + fuse + FFN, batched (M=32)."""
            Q1 = NQ + 1
            pt = pst.tile([P, 4, 2 * Q1], BF16, tag="tr")
            ctxlns = []
            for r in range(2):
                cps = ctx_ps_box.pop(r)
                stats = p2.tile([NQ, 6], F32, tag=f"stats2{r}")
                nc.vector.bn_stats(out=stats, in_=cps)
                mv = p2.tile([NQ, 2], F32, tag=f"mv2{r}")
                nc.vector.bn_aggr(out=mv, in_=stats)
                ve = p2.tile([NQ, 1], F32, tag=f"ve2{r}")
                nc.vector.tensor_scalar_add(out=ve, in0=mv[:, 1:2], scalar1=EPS)
                rstd = rsqrt(ve, NQ, f"p2{r}")
                ctxln = p2.tile([NQ, H], BF16, tag=f"ctxln{r}")
                if "gcln" in skip:
                    nc.vector.tensor_scalar(out=ctxln, in0=cps,
                                            scalar1=mv[:, 0:1], scalar2=rstd,
                                            op0=mybir.AluOpType.subtract,
                                            op1=mybir.AluOpType.mult)
                else:
                    cn = p2.tile([NQ, H], F32, tag=f"cn{r}")
                    nc.vector.tensor_scalar(out=cn, in0=cps,
                                            scalar1=mv[:, 0:1], scalar2=rstd,
                                            op0=mybir.AluOpType.subtract,
                                            op1=mybir.AluOpType.mult)
                    cgn = p2.tile([NQ, H], F32, tag=f"cgn{r}")
                    nc.vector.tensor_mul(out=cgn, in0=cn, in1=cg_sb)
                    nc.vector.tensor_add(out=ctxln, in0=cgn, in1=cb_sb)
                ctxlns.append(ctxln)
                # transpose this row's 4 chunks right away (pipelines with
                # the other row's DVE chain); row r lands at free offset r*Q1
                for j in range(4):
                    nc.tensor.transpose(pt[:, j, r * Q1:r * Q1 + NQ],
                                        ctxln[:, j * P:(j + 1) * P],
                                        idm_sb[0:NQ, M2:M2 + NQ])

            gcol = p2.tile([P, 4, 2], F32, tag="gcol")
            nc.vector.tensor_copy(out=gcol[:, :, 0:1], in_=pt[:, :, 0:1])
            nc.vector.tensor_copy(out=gcol[:, :, 1:2], in_=pt[:, :, Q1:Q1 + 1])
            fusedT = p2.tile([P, 8, M2], BF16, tag="fusedT")
            nc.vector.tensor_copy(out=fusedT[:, 0:4, 0:D], in_=pt[:, :, 1:1 + D])
            nc.vector.tensor_copy(out=fusedT[:, 0:4, D:M2],
                                  in_=pt[:, :, Q1 + 1:Q1 + 1 + D])
            for c in range(4):
                nc.vector.tensor_scalar_mul(out=fusedT[:, 4 + c, 0:D],
                                            in0=ones_col,
                                            scalar1=gcol[:, c, 0:1])
                nc.vector.tensor_scalar_mul(out=fusedT[:, 4 + c, D:M2],
                                            in0=ones_col,
                                            scalar1=gcol[:, c, 1:2])

            ph1 = ffnp.tile([M2, H], F32, tag="ph")
            for kc in range(8):
                nc.tensor.matmul(ph1, lhsT=fusedT[:, kc, :],
                                 rhs=w1_sb[:, kc, :], start=(kc == 0),
                                 stop=(kc == 7 and "bf1" in skip))
            if "bf1" not in skip:
                nc.tensor.matmul(ph1, lhsT=ones_row[:, :M2], rhs=bf1_sb,
                                 start=False, stop=True)
            h1 = p2.tile([M2, H], F32, tag="h1")
            nc.scalar.activation(out=h1, in_=ph1,
                                 func=mybir.ActivationFunctionType.Gelu)

            stats3 = p2.tile([M2, 6], F32, tag="stats3")
            nc.vector.bn_stats(out=stats3, in_=h1)
            mv3 = p2.tile([M2, 2], F32, tag="mv3")
            nc.vector.bn_aggr(out=mv3, in_=stats3)
            ve3 = p2.tile([M2, 1], F32, tag="ve3")
            nc.vector.tensor_scalar_add(out=ve3, in0=mv3[:, 1:2], scalar1=EPS)
            rstd3 = rsqrt(ve3, M2, "p3")
            h1ln = p2.tile([M2, H], BF16, tag="h1ln")
            if "fln" in skip:
                nc.vector.tensor_scalar(out=h1ln, in0=h1, scalar1=mv3[:, 0:1],
                                        scalar2=rstd3,
                                        op0=mybir.AluOpType.subtract,
                                        op1=mybir.AluOpType.mult)
            else:
                hn = p2.tile([M2, H], F32, tag="hn")
                nc.vector.tensor_scalar(out=hn, in0=h1, scalar1=mv3[:, 0:1],
                                        scalar2=rstd3,
                                        op0=mybir.AluOpType.subtract,
                                        op1=mybir.AluOpType.mult)
                hg = p2.tile([M2, H], F32, tag="hg")
                nc.vector.tensor_mul(out=hg, in0=hn, in1=fg_sb)
                nc.vector.tensor_add(out=h1ln, in0=hg, in1=fb_sb)

            pt2 = pst.tile([P, 4, M2], BF16, tag="tr")
            for j in range(4):
                nc.tensor.transpose(pt2[:, j, :], h1ln[:, j * P:(j + 1) * P],
                                    idm_sb[0:M2, 0:M2])
            h1T = p2.tile([P, 4, M2], BF16, tag="h1T")
            nc.vector.tensor_copy(out=h1T, in_=pt2)

            ph2 = ffnp.tile([M2, H], F32, tag="ph")
            for kc in range(4):
                nc.tensor.matmul(ph2, lhsT=h1T[:, kc, :], rhs=w2_sb[:, kc, :],
                                 start=(kc == 0),
                                 stop=(kc == 3 and "bf2" in skip))
            if "bf2" not in skip:
                nc.tensor.matmul(ph2, lhsT=ones_row[:, :M2], rhs=bf2_sb,
                                 start=False, stop=True)
            h2 = p2.tile([M2, H], F32, tag="h2")
            nc.scalar.activation(out=h2, in_=ph2,
                                 func=mybir.ActivationFunctionType.Gelu)

            prod = p2.tile([M2, H], F32, tag="prod")
            nc.vector.tensor_mul(out=prod, in0=h2, in1=wo_sb)
            lsum = p2.tile([M2, 1], F32, tag="lsum")
            nc.vector.reduce_sum(out=lsum, in_=prod, axis=mybir.AxisListType.X)
            nc.vector.tensor_scalar_add(out=logit_sb, in0=lsum,
                                        scalar1=float(b_out_s))

        # ---- driver ----
        # phase A: tiles 0,1 chunk-major, tile1 lagging LAG chunks so that
        # post(0) overlaps tile1's last chunks and frees px0 before proj(2)
        tsT0, px0 = proj_alloc(0)
        tsT1, px1 = proj_alloc(1)
        for k in range(KC + LAG):
            if k < KC:
                proj_chunk(tsT0, px0, k)
            if k >= LAG:
                proj_chunk(tsT1, px1, k - LAG)
        proj_bias(px0)
        proj_bias(px1)
        post(0)
        post(1)

        # steady stream: ctx matmuls trail the projection by two tiles
        # (slack for the exp chain)
        for t in range(2, TT):
            proj(t)
            if t + 4 < TT:
                dma_tile(t + 4)
            post(t)
            ctx_mm(t - 2)
            if t == TT - 1:
                # ACT switches Exp->Gelu here, hidden under the ctx drain
                nc.scalar.activation(out=dummy_act, in_=ones_row[:, 0:4],
                                     func=mybir.ActivationFunctionType.Gelu)
        ctx_mm(TT - 2)
        ctx_mm(TT - 1)
        tail()

        nc.sync.dma_start(out=out.ap(), in_=logit_sb)


def _np(x):
    return np.asarray(x)


LAST_RESULT = None


def kernel(**inputs):
    from concourse.bass_utils import run_bass_kernel_spmd

    token_states = _np(inputs["token_states"]).astype(np.float32)
    mask = _np(inputs["attention_mask"])
    W_proj = _np(inputs["W_proj"]).astype(np.float32)
    b_proj = _np(inputs["b_proj"]).astype(np.float32)
    tln_g = _np(inputs["tln_g"]).astype(np.float32)
    tln_b = _np(inputs["tln_b"]).astype(np.float32)
    gln_g = _np(inputs["gln_g"]).astype(np.float32)
    gln_b = _np(inputs["gln_b"]).astype(np.float32)
    cln_g = _np(inputs["cln_g"]).astype(np.float32)
    cln_b = _np(inputs["cln_b"]).astype(np.float32)
    fln_g = _np(inputs["fln_g"]).astype(np.float32)
    fln_b = _np(inputs["fln_b"]).astype(np.float32)
    domain_queries = _np(inputs["domain_queries"]).astype(np.float32)
    global_query = _np(inputs["global_query"]).astype(np.float32)
    W_ff1 = _np(inputs["W_ff1"]).astype(np.float32)
    b_ff1 = _np(inputs["b_ff1"]).astype(np.float32)
    W_ff2 = _np(inputs["W_ff2"]).astype(np.float32)
    b_ff2 = _np(inputs["b_ff2"]).astype(np.float32)
    W_out = _np(inputs["W_out"]).astype(np.float32)
    b_out = _np(inputs["b_out"]).astype(np.float32)
    log_temperature = _np(inputs["log_temperature"]).astype(np.float32)

    Bq, Sq = mask.shape
    assert (Bq, Sq) == (B, S) and token_states.shape == (B, S, TOKD)

    # ---- host preprocessing: joint per-core packing ----
    mb = mask.astype(bool)
    n_row = mb.sum(axis=1)                              # [B]
    n0 = n_row[0::2]                                    # per-core row0 count
    n1 = n_row[1::2]
    joint = n0 + n1
    TT = int(max(2, -(-int(joint.max()) // P)))
    T0E = int(max(1, -(-int(n0.max()) // P)))           # row0 tiles: [0, T0E)
    T1S = int(min(max(0, int(n0.min()) // P), TT - 1))  # row1 tiles: [T1S, TT)
    T0E = max(T0E, T1S + 1)
    NOV = T0E - T1S
    S_c = TT * P

    ts_c = np.zeros((N_CORES, S_c, TOKD), np.float32)
    # pb views: column t<T0E = row0 view, t>=T0E = row1 view,
    # columns TT+(t-T1S) = row1 view of overlap tiles
    pb_all = np.full((N_CORES, S_c, 2), -1e9, np.float32)  # [.., 0]=row0 view
    for c in range(N_CORES):
        i0 = np.flatnonzero(mb[2 * c])
        i1 = np.flatnonzero(mb[2 * c + 1])
        a, b_ = len(i0), len(i1)
        ts_c[c, :a] = token_states[2 * c, i0]
        ts_c[c, a:a + b_] = token_states[2 * c + 1, i1]
        pb_all[c, :a, 0] = 0.0
        pb_all[c, a:a + b_, 1] = 0.0

    temp = float(np.clip(np.exp(log_temperature[0]), 0.3, 3.0))
    inv_t = 1.0 / temp
    wo_host = (W_out[:, 0] * inv_t).astype(np.float32)
    b_out_s = float(b_out[0] * inv_t)

    # queries folded into the projection: row 0 = global, 1..16 = domains
    q_all = np.concatenate([global_query[None, :], domain_queries], 0)  # [17,H]
    q_eff = q_all * tln_g[None, :]                                      # [17,H]
    sq = q_eff.sum(axis=1)                                              # [17]
    w_bar = W_proj.mean(axis=1)                                         # [TOKD]
    Q_hat = W_proj @ q_eff.T - w_bar[:, None] * sq[None, :]             # [TOKD,17]
    # pre-rstd per-query score offset from the projection bias. (The LN-bias
    # term tln_b.q is a post-rstd per-query constant — a uniform softmax
    # rescale per query — and cancels in the context LN, so it's dropped.)
    bq = (b_proj @ q_eff.T) - float(b_proj.mean()) * sq

    W_aug = np.concatenate([W_proj, Q_hat], axis=1)                 # [TOKD,529]
    w_host = W_aug.reshape(KC, P, WTOT).transpose(1, 0, 2)          # [128,KC,529]
    bprow_host = np.concatenate([b_proj, bq]).reshape(1, WTOT)

    cg_host = np.ones((NQ, H), np.float32)
    cb_host = np.zeros((NQ, H), np.float32)
    cg_host[0] = gln_g
    cb_host[0] = gln_b
    cg_host[1:] = cln_g
    cb_host[1:] = cln_b

    bf16 = ml_dtypes.bfloat16

    idm_host = np.zeros((2 * D, 2 * D + NQ), np.float32)
    idm_host[:, 0:2 * D] = np.eye(2 * D)
    idm_host[0:NQ, 2 * D:] = np.eye(NQ)

    skip = set()
    if np.all(tln_g == 1) and np.all(tln_b == 0):
        skip.add("tln")
    if np.all(gln_g == 1) and np.all(gln_b == 0) \
            and np.all(cln_g == 1) and np.all(cln_b == 0):
        skip.add("gcln")
    if np.all(fln_g == 1) and np.all(fln_b == 0):
        skip.add("fln")
    if np.all(b_proj == 0) and np.all(bq == 0):
        skip.add("bp")
    if np.all(b_ff1 == 0):
        skip.add("bf1")
    if np.all(b_ff2 == 0):
        skip.add("bf2")

    nc = build_nc(TT, T1S, T0E, b_out_s, frozenset(skip))

    shared = dict(
        wp=w_host.reshape(P, KC * WTOT).astype(bf16),
        bprow=bprow_host.astype(bf16),
        tg=tln_g[None, :], tb=tln_b[None, :],
        cg=cg_host, cb=cb_host,
        fg=fln_g[None, :], fb=fln_b[None, :],
        w1=W_ff1.reshape(8, P, H).transpose(1, 0, 2).reshape(P, 8 * H).astype(bf16),
        bf1=b_ff1[None, :].astype(bf16),
        w2=W_ff2.reshape(4, P, H).transpose(1, 0, 2).reshape(P, 4 * H).astype(bf16),
        bf2=b_ff2[None, :].astype(bf16),
        wo=wo_host[None, :],
        idm=idm_host.astype(bf16),
    )

    in_maps = []
    for c in range(N_CORES):
        m = dict(shared)
        tsc = ts_c[c].reshape(TT, P, KC, P)           # [tile, s, c, p]
        m["ts"] = np.ascontiguousarray(
            tsc.transpose(0, 3, 2, 1)).reshape(TT * P, TOKD).astype(bf16)
        # pb columns: [P, TT + NOV]
        pbc = np.empty((P, TT + NOV), np.float32)
        v0 = pb_all[c, :, 0].reshape(TT, P).T          # row0 view per tile
        v1 = pb_all[c, :, 1].reshape(TT, P).T          # row1 view per tile
        pbc[:, :T0E] = v0[:, :T0E]
        pbc[:, T0E:TT] = v1[:, T0E:TT]
        pbc[:, TT:TT + NOV] = v1[:, T1S:T0E]
        m["pb"] = np.ascontiguousarray(pbc)
        in_maps.append(m)

    trace = os.environ.get("KERNEL_TRACE", "0") == "1"
    kw = {}
    if trace:
        kw = dict(trace=True, tmpdir=os.environ.get("KERNEL_TRACE_DIR") or None)
    res = run_bass_kernel_spmd(nc, in_maps, core_ids=list(range(N_CORES)), **kw)
    global LAST_RESULT
    LAST_RESULT = res
    outs = [res.results[c]["out"].reshape(B_LOCAL, D) for c in range(N_CORES)]
    return np.concatenate(outs, axis=0).astype(np.float32)


if __name__ == "__main__":
    pass


# revision 12
# speedup vs baseline: 1.0708x; 1.0708x over previous
"""Trainium2 Bass kernel for nn_MetaRouter (dense_transformer).

Contract: kernel(**inputs) takes FULL unsharded inputs (as produced by
reference.setup_inputs()) and returns the FULL [B, D] logits, matching
reference.reference(**inputs).

Strategy:
  - Data-parallel over batch: B=16 split as 2 rows per core x 8 cores.
    All parameters replicated. No collectives.
  - Host side: tokens with attention_mask==0 get softmax weight exactly 0
    for every query, so BOTH of a core's rows are compacted into ONE
    contiguous token stream (row0 tokens, then row1 tokens), padded to a
    multiple of 128 ("joint packing": ~17 tiles instead of 2x9).  Tiles
    that straddle the row boundary get TWO exp-weight vectors (one per
    row view, via per-view -1e9 pad biases), so each row's context
    accumulates only its own tokens.  ts is pre-cast to bf16 and
    pre-transposed into [tile, 128 feat, tok] chunks so the chip never
    transposes it.  The 17 attention queries are folded into the
    projection weight matrix as extra columns:
        Q_hat = W @ q - w_bar * colsum(q)   (w_bar = row-mean of W)
    which makes raw_score[s,q] = ts_s . Q_hat[:,q] = v_s.q - mu_s*sum(q),
    i.e. the LN mean-correction is pre-applied; only the per-token rstd
    scaling remains.  So scores cost 17 extra matmul columns, not a
    separate pass, and x^T never needs to exist on chip.
  - Softmax denominators and the LN mean shift both cancel inside the
    downstream LayerNorms (LN is invariant to positive scaling and
    uniform shifts), so the context sums use unnormalized exp weights
    against the raw (pre-LN) projections, with the per-token rstd folded
    into the exp weights.  No reduce_max, no reciprocal, no renorm.
  - Per tile of 128 tokens: 64 matmuls (32 k-chunks x 2 PSUM splits of
    256+273 columns; one PSUM bank each, LDWEIGHTS fully hidden), then
    DVE does bn_stats/bn_aggr + a bitcast-Newton rsqrt (keeps the ACT
    table pinned on Exp), ACT does exp(rstd*raw + padbias) in a single
    fused instruction, and the row contexts accumulate incrementally in
    one PSUM bank (row0 at partitions 0:17, row1 at 32:49 via the PE
    column-group mechanism) interleaved into the projection stream.
  - Startup: the first ~8us (engine boot + DMA bring-up) are covered by
    garbage warm-up matmuls that also pre-warm the PE HAM clock gate;
    tiles 0 and 1 are then co-processed CHUNK-MAJOR (tile1 lagging six
    k-chunks) so each arriving W' chunk feeds two tiles' matmuls --
    this halves the W'-bandwidth-per-flop while W' (4.3MB) streams in.
    W' and the first two ts tiles are striped across all three DMA
    queues (2 HWDGE + SWDGE) in consumption order.
  - Tail: both rows' context LN + fuse + FFN run ONCE batched (M=32
    instead of 2xM=16): stats via bn_stats on the two partition groups,
    bitcast-Newton rsqrt, PE transposes (row groups 0/32 run
    concurrently), gelu-FFN with all weights pre-chunked, output head
    folded with the temperature on the host.  ACT function tables (Exp,
    Gelu) are preloaded off the critical path by dummy activations.
"""

import os

import numpy as np
import ml_dtypes

import concourse.bass as bass
import concourse.bacc as bacc
import concourse.tile as tile
from concourse import mybir

P = 128
H = 512
TOKD = 4096
KC = TOKD // P    # 32 k-chunks of the projection contraction
NQ = 17           # 1 global + 16 domain queries
WTOT = H + NQ     # 529 projection output columns
SPL = 256         # W column split; region B holds W[256:] + scores
D = 16
B = 16
S = 2048
N_CORES = 8
B_LOCAL = B // N_CORES
EPS = 1e-5
F32 = mybir.dt.float32
I32 = mybir.dt.int32
BF16 = mybir.dt.bfloat16
MAGIC = 0x5F3759DF
LAG = 6           # tile1 k-chunk lag during the startup co-processing
NWARM = 22        # garbage warm-up matmuls covering engine/DMA boot


def build_nc(TT: int, T1S: int, T0E: int, b_out_s: float, skip=frozenset()):
    """Per-core Bass program: TT jointly-packed tiles, row0 ctx over tiles
    [0,T0E), row1 over [T1S,TT); tiles in [T1S,T0E) carry two exp views."""
    NOV = T0E - T1S

    nc = bacc.Bacc("TRN2", target_bir_lowering=False, num_swdge_queues=1)

    ts = nc.declare_dram_parameter("ts", [TT * P, TOKD], BF16, isOutput=False)
    pb = nc.declare_dram_parameter("pb", [P, TT + NOV], F32, isOutput=False)
    wp = nc.declare_dram_parameter("wp", [P, KC * WTOT], BF16, isOutput=False)
    bprow = nc.declare_dram_parameter("bprow", [1, WTOT], BF16, isOutput=False)
    tg = nc.declare_dram_parameter("tg", [1, H], F32, isOutput=False)
    tb = nc.declare_dram_parameter("tb", [1, H], F32, isOutput=False)
    cg = nc.declare_dram_parameter("cg", [NQ, H], F32, isOutput=False)
    cb = nc.declare_dram_parameter("cb", [NQ, H], F32, isOutput=False)
    fg = nc.declare_dram_parameter("fg", [1, H], F32, isOutput=False)
    fb = nc.declare_dram_parameter("fb", [1, H], F32, isOutput=False)
    w1 = nc.declare_dram_parameter("w1", [P, 8 * H], BF16, isOutput=False)
    bf1 = nc.declare_dram_parameter("bf1", [1, H], BF16, isOutput=False)
    w2 = nc.declare_dram_parameter("w2", [P, 4 * H], BF16, isOutput=False)
    bf2 = nc.declare_dram_parameter("bf2", [1, H], BF16, isOutput=False)
    wo = nc.declare_dram_parameter("wo", [1, H], F32, isOutput=False)
    idm = nc.declare_dram_parameter("idm", [2 * D, 2 * D + NQ], BF16,
                                    isOutput=False)
    out = nc.declare_dram_parameter("out", [B_LOCAL * D, 1], F32, isOutput=True)

    with tile.TileContext(nc) as tc:
        _emit(tc, nc, TT, T1S, T0E, b_out_s, skip,
              ts=ts, pb=pb, wp=wp, bprow=bprow, tg=tg, tb=tb, cg=cg, cb=cb,
              fg=fg, fb=fb, w1=w1, bf1=bf1, w2=w2, bf2=bf2, wo=wo, idm=idm,
              out=out)
    nc.compile()
    return nc


def _emit(tc, nc, TT, T1S, T0E, b_out_s, skip, *, ts, pb, wp, bprow, tg, tb,
          cg, cb, fg, fb, w1, bf1, w2, bf2, wo, idm, out):
    from contextlib import ExitStack
    NOV = T0E - T1S
    M2 = 2 * D        # both rows' FFN batched: 32 output rows
    R1 = 32           # row1 ctx partition base (PE column-group aligned)
    ctx = ExitStack()
    with ctx:
        const = ctx.enter_context(tc.tile_pool(name="const", bufs=1))
        tsp = ctx.enter_context(tc.tile_pool(name="tsp", bufs=6))
        xp = ctx.enter_context(tc.tile_pool(name="xp", bufs=1))
        lnp = ctx.enter_context(tc.tile_pool(name="lnp", bufs=2))
        p2 = ctx.enter_context(tc.tile_pool(name="p2", bufs=2))
        psx = ctx.enter_context(tc.tile_pool(name="psx", bufs=2, space="PSUM"))
        ctxp = ctx.enter_context(tc.tile_pool(name="ctxp", bufs=1, space="PSUM"))
        pst = ctx.enter_context(tc.tile_pool(name="pst", bufs=1, space="PSUM"))
        ffnp = ctx.enter_context(tc.tile_pool(name="ffnp", bufs=1, space="PSUM"))

        # ---- SBUF-resident state ----
        x_sb = xp.tile([P, TT, H], BF16)
        pexpT = xp.tile([P, TT + NOV, NQ], BF16)
        logit_sb = xp.tile([M2, 1], F32)

        # ---- cheap constants + PE warm-up (no DMA dependencies) ----
        ones_row = const.tile([1, P], BF16)
        nc.vector.memset(ones_row, 1.0)
        ones_col = const.tile([P, D], BF16)
        nc.vector.memset(ones_col, 1.0)
        magic = const.tile([P, 1], I32)
        nc.vector.memset(magic, MAGIC)
        # warm-up operand must be initialized (race detector) and cheap
        nc.vector.memset(x_sb[:, 0, :], 0.0)

        # garbage matmuls: occupy the PE through engine/DMA bring-up so the
        # HAM clock-gate is at 8/8 before real work, and the PE never sees
        # a >3.4us idle window at the start.
        wtile = psx.tile([P, 2 * H], F32, tag="px", name="warm")
        for _ in range(NWARM):
            nc.tensor.matmul(wtile[:, 0:SPL], lhsT=x_sb[:, 0, 0:P],
                             rhs=x_sb[:, 0, 0:SPL], start=True, stop=True,
                             skip_group_check=True)

        # identities come from the host (idm): [0:32,0:32]=I32 for the FFN
        # transposes; [0:17,32:49] and [32:49,32:49]=I17 for the two ctx
        # transpose row-groups.
        idm_sb = const.tile([M2, M2 + NQ], BF16)

        # ---- startup DMA stream, striped across the three queues in
        # consumption order (phase A consumes W chunk-major over 2 tiles).
        w_sb = const.tile([P, KC, WTOT], BF16)
        _wp = wp.ap().rearrange("p (c w) -> p c w", w=WTOT)
        pb_sb = const.tile([P, TT + NOV], F32)
        nc.sync.dma_start(out=pb_sb, in_=pb.ap())

        prefetched = {}

        def dma_tile(t, eng=None):
            if eng is None:
                eng = (nc.gpsimd, nc.sync, nc.scalar)[t % 3]
            tt = tsp.tile([P, KC * P], BF16, tag="ts")
            eng.dma_start(out=tt, in_=ts.ap()[t * P:(t + 1) * P, :])
            prefetched[t] = tt

        def dma_tile_blocks(t, eng_list):
            """4 block-DMAs of 8 k-chunks each (2KB lines) for one tile."""
            tt = tsp.tile([P, KC * P], BF16, tag="ts")
            tv = tt.rearrange("p (c s) -> p c s", s=P)
            sv = ts.ap()[t * P:(t + 1) * P, :].rearrange("p (c s) -> p c s", s=P)
            for b, eng in enumerate(eng_list):
                eng.dma_start(out=tv[:, 8 * b:8 * b + 8, :],
                              in_=sv[:, 8 * b:8 * b + 8, :])
            prefetched[t] = tt

        def dma_w(q, eng):
            """One 4-chunk block of W' (~0.54MB, 8.5KB lines)."""
            eng.dma_start(out=w_sb[:, 4 * q:4 * q + 4, :],
                          in_=_wp[:, 4 * q:4 * q + 4, :])

        # need order: Wq0,ts0b0 | Wq1 | ts1b0 | Wq2,ts0b1 | Wq3 | ts1b1 |
        #             Wq4,ts0b2 | Wq5 | ts1b2 | Wq6,ts0b3 | Wq7 | ts1b3
        # sync/scalar (HWDGE) lead; gpsimd (SWDGE) boots ~3us later so it
        # carries items needed from mid-phase-A on.
        dma_w(0, nc.sync)
        ts0 = tsp.tile([P, KC * P], BF16, tag="ts", name="ts0")
        t0v = ts0.rearrange("p (c s) -> p c s", s=P)
        s0v = ts.ap()[0:P, :].rearrange("p (c s) -> p c s", s=P)
        ts1 = tsp.tile([P, KC * P], BF16, tag="ts", name="ts1")
        t1v = ts1.rearrange("p (c s) -> p c s", s=P)
        s1v = ts.ap()[P:2 * P, :].rearrange("p (c s) -> p c s", s=P)
        prefetched[0] = ts0
        prefetched[1] = ts1

        nc.scalar.dma_start(out=t0v[:, 0:8, :], in_=s0v[:, 0:8, :])   # ts0.b0
        dma_w(1, nc.scalar)
        nc.sync.dma_start(out=t1v[:, 0:8, :], in_=s1v[:, 0:8, :])     # ts1.b0
        dma_w(2, nc.scalar)
        nc.gpsimd.dma_start(out=t0v[:, 8:16, :], in_=s0v[:, 8:16, :])  # ts0.b1
        dma_w(3, nc.sync)
        nc.scalar.dma_start(out=t1v[:, 8:16, :], in_=s1v[:, 8:16, :])  # ts1.b1
        dma_w(4, nc.gpsimd)
        nc.sync.dma_start(out=t0v[:, 16:24, :], in_=s0v[:, 16:24, :])  # ts0.b2
        dma_w(5, nc.scalar)
        nc.gpsimd.dma_start(out=t1v[:, 16:24, :], in_=s1v[:, 16:24, :])  # ts1.b2
        dma_w(6, nc.sync)
        nc.scalar.dma_start(out=t0v[:, 24:32, :], in_=s0v[:, 24:32, :])  # ts0.b3
        dma_w(7, nc.gpsimd)
        nc.sync.dma_start(out=t1v[:, 24:32, :], in_=s1v[:, 24:32, :])  # ts1.b3
        # next tiles land behind the startup stream, block-striped so tile 2/3
        # compute can start on partial arrivals (the stream is DMA-paced
        # until ~tile 3)
        dma_tile_blocks(2, (nc.sync, nc.scalar, nc.gpsimd, nc.sync))
        dma_tile_blocks(3, (nc.scalar, nc.gpsimd, nc.sync, nc.scalar))
        dma_tile(4, nc.gpsimd)
        dma_tile(5, nc.scalar)

        # ---- ACT table preload: first Exp use must not pay the table load
        dummy_act = lnp.tile([1, 4], F32, tag="dummy_act")
        nc.scalar.activation(out=dummy_act, in_=ones_row[:, 0:4],
                             func=mybir.ActivationFunctionType.Exp)

        # ---- tail constants (DMA-idle region; needed only at the tail) ----
        def bcast(dram, parts, dt=F32):
            t = const.tile([parts, H], dt, tag=f"c_{dram.name}")
            a = dram.ap()
            nc.sync.dma_start(
                out=t, in_=bass.AP(tensor=a.tensor, offset=a.offset,
                                   ap=[[0, parts]] + list(a.ap[1:])))
            return t

        w1_sb = const.tile([P, 8, H], BF16)
        nc.sync.dma_start(out=w1_sb, in_=w1.ap().rearrange("p (c h) -> p c h", h=H))
        w2_sb = const.tile([P, 4, H], BF16)
        nc.scalar.dma_start(out=w2_sb, in_=w2.ap().rearrange("p (c h) -> p c h", h=H))
        wo_sb = bcast(wo, M2)
        if "tln" not in skip:
            tg_sb = bcast(tg, P)
            tb_sb = bcast(tb, P)
        if "gcln" not in skip:
            cg_sb = const.tile([NQ, H], F32)
            nc.scalar.dma_start(out=cg_sb, in_=cg.ap())
            cb_sb = const.tile([NQ, H], F32)
            nc.scalar.dma_start(out=cb_sb, in_=cb.ap())
        if "fln" not in skip:
            fg_sb = bcast(fg, M2)
            fb_sb = bcast(fb, M2)
        if "bf1" not in skip:
            bf1_sb = const.tile([1, H], BF16)
            nc.sync.dma_start(out=bf1_sb, in_=bf1.ap())
        if "bf2" not in skip:
            bf2_sb = const.tile([1, H], BF16)
            nc.sync.dma_start(out=bf2_sb, in_=bf2.ap())
        if "bp" not in skip:
            bprow_sb = const.tile([1, WTOT], BF16)
            nc.sync.dma_start(out=bprow_sb, in_=bprow.ap())
        nc.sync.dma_start(out=idm_sb, in_=idm.ap())

        def rsqrt(ve, parts, tag, iters=1):
            """y ~= (ve)^-0.5 via bitcast seed + Newton steps (DVE only)."""
            y = lnp.tile([parts, 1], F32, tag=f"y_{tag}")
            sh = lnp.tile([parts, 1], I32, tag=f"sh_{tag}")
            nc.vector.tensor_scalar(out=sh, in0=ve.bitcast(I32), scalar1=1,
                                    scalar2=None,
                                    op0=mybir.AluOpType.arith_shift_right)
            nc.vector.tensor_tensor(out=y.bitcast(I32), in0=magic[:parts],
                                    in1=sh, op=mybir.AluOpType.subtract)
            t1 = lnp.tile([parts, 1], F32, tag=f"t1_{tag}")
            hh = lnp.tile([parts, 1], F32, tag=f"h_{tag}")
            for _ in range(iters):
                nc.vector.tensor_mul(out=t1, in0=y, in1=y)
                nc.vector.tensor_mul(out=t1, in0=t1, in1=ve)
                nc.vector.tensor_scalar(out=hh, in0=t1, scalar1=-0.5,
                                        scalar2=1.5, op0=mybir.AluOpType.mult,
                                        op1=mybir.AluOpType.add)
                nc.vector.tensor_mul(out=y, in0=y, in1=hh)
            return y

        psums = {}

        def proj_alloc(t):
            tsT = prefetched.pop(t).rearrange("p (c s) -> p c s", s=P)
            px = psx.tile([P, 2 * H], F32, tag="px", name="px")
            psums[t] = px
            return tsT, px

        def proj_chunk(tsT, px, k):
            first, last = k == 0, k == KC - 1 and "bp" in skip
            nc.tensor.matmul(px[:, 0:SPL], lhsT=tsT[:, k, :],
                             rhs=w_sb[:, k, 0:SPL], start=first, stop=last,
                             skip_group_check=True)
            nc.tensor.matmul(px[:, H:H + WTOT - SPL], lhsT=tsT[:, k, :],
                             rhs=w_sb[:, k, SPL:], start=first, stop=last,
                             skip_group_check=True)

        def proj_bias(px):
            if "bp" not in skip:
                nc.tensor.matmul(px[:, 0:SPL], lhsT=ones_row,
                                 rhs=bprow_sb[:, 0:SPL], start=False,
                                 stop=True, skip_group_check=True)
                nc.tensor.matmul(px[:, H:H + WTOT - SPL], lhsT=ones_row,
                                 rhs=bprow_sb[:, SPL:], start=False,
                                 stop=True, skip_group_check=True)

        def proj(t):
            tsT, px = proj_alloc(t)
            for k in range(KC):
                proj_chunk(tsT, px, k)
            proj_bias(px)

        def exp_slots(t):
            """pexpT/pb column(s) for tile t: primary view + overlap view."""
            slots = [t]
            if T1S <= t < T0E:
                slots.append(TT + (t - T1S))
            return slots

        def post(t):
            """Stats + x store + exp-weights for tile t (DVE/ACT work)."""
            px = psums.pop(t)
            vreg = px.rearrange("p (r x) -> p r x", x=H)[:, :, 0:SPL]
            stats = lnp.tile([P, 12], F32, tag="stats")
            nc.vector.bn_stats(out=stats[:, 0:6], in_=vreg[:, 0, :])
            nc.vector.bn_stats(out=stats[:, 6:12], in_=vreg[:, 1, :])
            mv = lnp.tile([P, 2], F32, tag="mv")
            nc.vector.bn_aggr(out=mv, in_=stats)
            ve = lnp.tile([P, 1], F32, tag="ve")
            nc.vector.tensor_scalar_add(out=ve, in0=mv[:, 1:2], scalar1=EPS)
            # when x stays raw ("tln" skipped), rstd only scales the softmax
            # weights per token; the bitcast seed's 3.4% per-token error
            # averages out across ~1k tokens in the context sums, so skip
            # the Newton step in that path.
            rstd = rsqrt(ve, P, "p1", iters=0 if "tln" in skip else 1)
            # attn weights first: they gate the trailing ctx matmul
            for sl in exp_slots(t):
                nc.scalar.activation(out=pexpT[:, sl, :],
                                     in_=px[:, H + H - SPL:H + WTOT - SPL],
                                     func=mybir.ActivationFunctionType.Exp,
                                     bias=pb_sb[:, sl:sl + 1], scale=rstd)
            xv = x_sb[:, t, :].rearrange("p (r x) -> p r x", x=SPL)
            if "tln" in skip:
                for sl in exp_slots(t):
                    nc.vector.tensor_scalar_mul(out=pexpT[:, sl, :],
                                                in0=pexpT[:, sl, :],
                                                scalar1=rstd)
                # store raw v; rstd folds into the attn weights, mu cancels
                nc.vector.tensor_copy(out=xv, in_=vreg)
            else:
                xa = lnp.tile([P, H], F32, tag="xa")
                nc.vector.tensor_scalar(
                    out=xa.rearrange("p (r x) -> p r x", x=SPL), in0=vreg,
                    scalar1=mv[:, 0:1], scalar2=rstd,
                    op0=mybir.AluOpType.subtract, op1=mybir.AluOpType.mult)
                xg = lnp.tile([P, H], F32, tag="xg")
                nc.vector.tensor_mul(out=xg, in0=xa, in1=tg_sb)
                nc.vector.tensor_add(out=x_sb[:, t, :], in0=xg, in1=tb_sb)

        ctx_ps_box = {}

        def ctx_mm(u):
            """Accumulate row contexts for tile u (one PSUM bank per row,
            both at partition base 0)."""
            if u == 0:
                ctx_ps_box[0] = ctxp.tile([NQ, H], F32, tag="ctx0", name="ctx0")
                ctx_ps_box[1] = ctxp.tile([NQ, H], F32, tag="ctx1", name="ctx1")
            if u < T0E:
                nc.tensor.matmul(ctx_ps_box[0], lhsT=pexpT[:, u, :],
                                 rhs=x_sb[:, u, :], start=(u == 0),
                                 stop=(u == T0E - 1), skip_group_check=True)
            if u >= T1S:
                sl = TT + (u - T1S) if u < T0E else u
                nc.tensor.matmul(ctx_ps_box[1], lhsT=pexpT[:, sl, :],
                                 rhs=x_sb[:, u, :], start=(u == T1S),
                                 stop=(u == TT - 1), skip_group_check=True)

        def tail():
            """Both rows' ctx LN + fuse + FFN, batched (M=32)."""
            Q1 = NQ + 1
            pt = pst.tile([P, 4, 2 * Q1], BF16, tag="tr")
            ctxlns = []
            for r in range(2):
                cps = ctx_ps_box.pop(r)
                stats = p2.tile([NQ, 6], F32, tag=f"stats2{r}")
                nc.vector.bn_stats(out=stats, in_=cps)
                mv = p2.tile([NQ, 2], F32, tag=f"mv2{r}")
                nc.vector.bn_aggr(out=mv, in_=stats)
                ve = p2.tile([NQ, 1], F32, tag=f"ve2{r}")
                nc.vector.tensor_scalar_add(out=ve, in0=mv[:, 1:2], scalar1=EPS)
                rstd = rsqrt(ve, NQ, f"p2{r}")
                ctxln = p2.tile([NQ, H], BF16, tag=f"ctxln{r}")
                if "gcln" in skip:
                    nc.vector.tensor_scalar(out=ctxln, in0=cps,
                                            scalar1=mv[:, 0:1], scalar2=rstd,
                                            op0=mybir.AluOpType.subtract,
                                            op1=mybir.AluOpType.mult)
                else:
                    cn = p2.tile([NQ, H], F32, tag=f"cn{r}")
                    nc.vector.tensor_scalar(out=cn, in0=cps,
                                            scalar1=mv[:, 0:1], scalar2=rstd,
                                            op0=mybir.AluOpType.subtract,
                                            op1=mybir.AluOpType.mult)
                    cgn = p2.tile([NQ, H], F32, tag=f"cgn{r}")
                    nc.vector.tensor_mul(out=cgn, in0=cn, in1=cg_sb)
                    nc.vector.tensor_add(out=ctxln, in0=cgn, in1=cb_sb)
                ctxlns.append(ctxln)
                # transpose this row's 4 chunks right away (pipelines with
                # the other row's DVE chain); row r lands at free offset r*Q1
                for j in range(4):
                    nc.tensor.transpose(pt[:, j, r * Q1:r * Q1 + NQ],
                                        ctxln[:, j * P:(j + 1) * P],
                                        idm_sb[0:NQ, M2:M2 + NQ])

            gcol = p2.tile([P, 4, 2], F32, tag="gcol")
            nc.vector.tensor_copy(out=gcol[:, :, 0:1], in_=pt[:, :, 0:1])
            nc.vector.tensor_copy(out=gcol[:, :, 1:2], in_=pt[:, :, Q1:Q1 + 1])
            fusedT = p2.tile([P, 8, M2], BF16, tag="fusedT")
            nc.vector.tensor_copy(out=fusedT[:, 0:4, 0:D], in_=pt[:, :, 1:1 + D])
            nc.vector.tensor_copy(out=fusedT[:, 0:4, D:M2],
                                  in_=pt[:, :, Q1 + 1:Q1 + 1 + D])
            for c in range(4):
                nc.vector.tensor_scalar_mul(out=fusedT[:, 4 + c, 0:D],
                                            in0=ones_col,
                                            scalar1=gcol[:, c, 0:1])
                nc.vector.tensor_scalar_mul(out=fusedT[:, 4 + c, D:M2],
                                            in0=ones_col,
                                            scalar1=gcol[:, c, 1:2])

            ph1 = ffnp.tile([M2, H], F32, tag="ph")
            for kc in range(8):
                nc.tensor.matmul(ph1, lhsT=fusedT[:, kc, :],
                                 rhs=w1_sb[:, kc, :], start=(kc == 0),
                                 stop=(kc == 7 and "bf1" in skip))
            if "bf1" not in skip:
                nc.tensor.matmul(ph1, lhsT=ones_row[:, :M2], rhs=bf1_sb,
                                 start=False, stop=True)
            h1 = p2.tile([M2, H], F32, tag="h1")
            nc.scalar.activation(out=h1, in_=ph1,
                                 func=mybir.ActivationFunctionType.Gelu)

            stats3 = p2.tile([M2, 6], F32, tag="stats3")
            nc.vector.bn_stats(out=stats3, in_=h1)
            mv3 = p2.tile([M2, 2], F32, tag="mv3")
            nc.vector.bn_aggr(out=mv3, in_=stats3)
            ve3 = p2.tile([M2, 1], F32, tag="ve3")
            nc.vector.tensor_scalar_add(out=ve3, in0=mv3[:, 1:2], scalar1=EPS)
            rstd3 = rsqrt(ve3, M2, "p3")
            h1ln = p2.tile([M2, H], BF16, tag="h1ln")
            if "fln" in skip:
                nc.vector.tensor_scalar(out=h1ln, in0=h1, scalar1=mv3[:, 0:1],
                                        scalar2=rstd3,
                                        op0=mybir.AluOpType.subtract,
                                        op1=mybir.AluOpType.mult)
            else:
                hn = p2.tile([M2, H], F32, tag="hn")
                nc.vector.tensor_scalar(out=hn, in0=h1, scalar1=mv3[:, 0:1],
                                        scalar2=rstd3,
                                        op0=mybir.AluOpType.subtract,
                                        op1=mybir.AluOpType.mult)
                hg = p2.tile([M2, H], F32, tag="hg")
                nc.vector.tensor_mul(out=hg, in0=hn, in1=fg_sb)
                nc.vector.tensor_add(out=h1ln, in0=hg, in1=fb_sb)

            pt2 = pst.tile([P, 4, M2], BF16, tag="tr")
            for j in range(4):
                nc.tensor.transpose(pt2[:, j, :], h1ln[:, j * P:(j + 1) * P],
                                    idm_sb[0:M2, 0:M2])
            h1T = p2.tile([P, 4, M2], BF16, tag="h1T")
            nc.vector.tensor_copy(out=h1T, in_=pt2)

            ph2 = ffnp.tile([M2, H], F32, tag="ph")
            for kc in range(4):
                nc.tensor.matmul(ph2, lhsT=h1T[:, kc, :], rhs=w2_sb[:, kc, :],
                                 start=(kc == 0),
                                 stop=(kc == 3 and "bf2" in skip))
            if "bf2" not in skip:
                nc.tensor.matmul(ph2, lhsT=ones_row[:, :M2], rhs=bf2_sb,
                                 start=False, stop=True)
            h2 = p2.tile([M2, H], F32, tag="h2")
            nc.scalar.activation(out=h2, in_=ph2,
                                 func=mybir.ActivationFunctionType.Gelu)

            prod = p2.tile([M2, H], F32, tag="prod")
            nc.vector.tensor_mul(out=prod, in0=h2, in1=wo_sb)
            lsum = p2.tile([M2, 1], F32, tag="lsum")
            nc.vector.reduce_sum(out=lsum, in_=prod, axis=mybir.AxisListType.X)
            nc.vector.tensor_scalar_add(out=logit_sb, in0=lsum,
                                        scalar1=float(b_out_s))

        # ---- driver ----
        # phase A: tiles 0,1 chunk-major, tile1 lagging LAG chunks so that
        # post(0) overlaps tile1's last chunks and frees px0 before proj(2)
        tsT0, px0 = proj_alloc(0)
        tsT1, px1 = proj_alloc(1)
        for k in range(KC + LAG):
            if k < KC:
                proj_chunk(tsT0, px0, k)
            if k >= LAG:
                proj_chunk(tsT1, px1, k - LAG)
        proj_bias(px0)
        proj_bias(px1)
        post(0)
        post(1)

        # steady stream: ctx matmuls trail the projection by two tiles
        # (slack for the exp chain)
        for t in range(2, TT):
            proj(t)
            if t + 4 < TT:
                dma_tile(t + 4)
            post(t)
            ctx_mm(t - 2)
            if t == TT - 1:
                # ACT switches Exp->Gelu here, hidden under the ctx drain
                nc.scalar.activation(out=dummy_act, in_=ones_row[:, 0:4],
                                     func=mybir.ActivationFunctionType.Gelu)
        ctx_mm(TT - 2)
        ctx_mm(TT - 1)
        tail()

        nc.sync.dma_start(out=out.ap(), in_=logit_sb)


def _np(x):
    return np.asarray(x)


LAST_RESULT = None


def kernel(**inputs):
    from concourse.bass_utils import run_bass_kernel_spmd

    token_states = _np(inputs["token_states"]).astype(np.float32)
    mask = _np(inputs["attention_mask"])
    W_proj = _np(inputs["W_proj"]).astype(np.float32)
    b_proj = _np(inputs["b_proj"]).astype(np.float32)
    tln_g = _np(inputs["tln_g"]).astype(np.float32)
    tln_b = _np(inputs["tln_b"]).astype(np.float32)
    gln_g = _np(inputs["gln_g"]).astype(np.float32)
    gln_b = _np(inputs["gln_b"]).astype(np.float32)
    cln_g = _np(inputs["cln_g"]).astype(np.float32)
    cln_b = _np(inputs["cln_b"]).astype(np.float32)
    fln_g = _np(inputs["fln_g"]).astype(np.float32)
    fln_b = _np(inputs["fln_b"]).astype(np.float32)
    domain_queries = _np(inputs["domain_queries"]).astype(np.float32)
    global_query = _np(inputs["global_query"]).astype(np.float32)
    W_ff1 = _np(inputs["W_ff1"]).astype(np.float32)
    b_ff1 = _np(inputs["b_ff1"]).astype(np.float32)
    W_ff2 = _np(inputs["W_ff2"]).astype(np.float32)
    b_ff2 = _np(inputs["b_ff2"]).astype(np.float32)
    W_out = _np(inputs["W_out"]).astype(np.float32)
    b_out = _np(inputs["b_out"]).astype(np.float32)
    log_temperature = _np(inputs["log_temperature"]).astype(np.float32)

    Bq, Sq = mask.shape
    assert (Bq, Sq) == (B, S) and token_states.shape == (B, S, TOKD)

    # ---- host preprocessing: joint per-core packing ----
    mb = mask.astype(bool)
    n_row = mb.sum(axis=1)                              # [B]
    n0 = n_row[0::2]                                    # per-core row0 count
    n1 = n_row[1::2]
    joint = n0 + n1
    TT = int(max(2, -(-int(joint.max()) // P)))
    T0E = int(max(1, -(-int(n0.max()) // P)))           # row0 tiles: [0, T0E)
    T1S = int(min(max(0, int(n0.min()) // P), TT - 1))  # row1 tiles: [T1S, TT)
    T0E = max(T0E, T1S + 1)
    NOV = T0E - T1S
    S_c = TT * P

    ts_c = np.zeros((N_CORES, S_c, TOKD), np.float32)
    # pb views: column t<T0E = row0 view, t>=T0E = row1 view,
    # columns TT+(t-T1S) = row1 view of overlap tiles
    pb_all = np.full((N_CORES, S_c, 2), -1e9, np.float32)  # [.., 0]=row0 view
    for c in range(N_CORES):
        i0 = np.flatnonzero(mb[2 * c])
        i1 = np.flatnonzero(mb[2 * c + 1])
        a, b_ = len(i0), len(i1)
        ts_c[c, :a] = token_states[2 * c, i0]
        ts_c[c, a:a + b_] = token_states[2 * c + 1, i1]
        pb_all[c, :a, 0] = 0.0
        pb_all[c, a:a + b_, 1] = 0.0

    temp = float(np.clip(np.exp(log_temperature[0]), 0.3, 3.0))
    inv_t = 1.0 / temp
    wo_host = (W_out[:, 0] * inv_t).astype(np.float32)
    b_out_s = float(b_out[0] * inv_t)

    # queries folded into the projection: row 0 = global, 1..16 = domains
    q_all = np.concatenate([global_query[None, :], domain_queries], 0)  # [17,H]
    q_eff = q_all * tln_g[None, :]                                      # [17,H]
    sq = q_eff.sum(axis=1)                                              # [17]
    w_bar = W_proj.mean(axis=1)                                         # [TOKD]
    Q_hat = W_proj @ q_eff.T - w_bar[:, None] * sq[None, :]             # [TOKD,17]
    # pre-rstd per-query score offset from the projection bias. (The LN-bias
    # term tln_b.q is a post-rstd per-query constant — a uniform softmax
    # rescale per query — and cancels in the context LN, so it's dropped.)
    bq = (b_proj @ q_eff.T) - float(b_proj.mean()) * sq

    W_aug = np.concatenate([W_proj, Q_hat], axis=1)                 # [TOKD,529]
    w_host = W_aug.reshape(KC, P, WTOT).transpose(1, 0, 2)          # [128,KC,529]
    bprow_host = np.concatenate([b_proj, bq]).reshape(1, WTOT)

    cg_host = np.ones((NQ, H), np.float32)
    cb_host = np.zeros((NQ, H), np.float32)
    cg_host[0] = gln_g
    cb_host[0] = gln_b
    cg_host[1:] = cln_g
    cb_host[1:] = cln_b

    bf16 = ml_dtypes.bfloat16

    idm_host = np.zeros((2 * D, 2 * D + NQ), np.float32)
    idm_host[:, 0:2 * D] = np.eye(2 * D)
    idm_host[0:NQ, 2 * D:] = np.eye(NQ)

    skip = set()
    if np.all(tln_g == 1) and np.all(tln_b == 0):
        skip.add("tln")
    if np.all(gln_g == 1) and np.all(gln_b == 0) \
            and np.all(cln_g == 1) and np.all(cln_b == 0):
        skip.add("gcln")
    if np.all(fln_g == 1) and np.all(fln_b == 0):
        skip.add("fln")
    if np.all(b_proj == 0) and np.all(bq == 0):
        skip.add("bp")
    if np.all(b_ff1 == 0):
        skip.add("bf1")
    if np.all(b_ff2 == 0):
        skip.add("bf2")

    nc = build_nc(TT, T1S, T0E, b_out_s, frozenset(skip))

    shared = dict(
        wp=w_host.reshape(P, KC * WTOT).astype(bf16),
        bprow=bprow_host.astype(bf16),
        tg=tln_g[None, :], tb=tln_b[None, :],
        cg=cg_host, cb=cb_host,
        fg=fln_g[None, :], fb=fln_b[None, :],
        w1=W_ff1.reshape(8, P, H).transpose(1, 0, 2).reshape(P, 8 * H).astype(bf16),
        bf1=b_ff1[None, :].astype(bf16),
        w2=W_ff2.reshape(4, P, H).transpose(1, 0, 2).reshape(P, 4 * H).astype(bf16),
        bf2=b_ff2[None, :].astype(bf16),
        wo=wo_host[None, :],
        idm=idm_host.astype(bf16),
    )

    in_maps = []
    for c in range(N_CORES):
        m = dict(shared)
        tsc = ts_c[c].reshape(TT, P, KC, P)           # [tile, s, c, p]
        m["ts"] = np.ascontiguousarray(
            tsc.transpose(0, 3, 2, 1)).reshape(TT * P, TOKD).astype(bf16)
        # pb columns: [P, TT + NOV]
        pbc = np.empty((P, TT + NOV), np.float32)
        v0 = pb_all[c, :, 0].reshape(TT, P).T          # row0 view per tile
        v1 = pb_all[c, :, 1].reshape(TT, P).T          # row1 view per tile
        pbc[:, :T0E] = v0[:, :T0E]
        pbc[:, T0E:TT] = v1[:, T0E:TT]
        pbc[:, TT:TT + NOV] = v1[:, T1S:T0E]
        m["pb"] = np.ascontiguousarray(pbc)
        in_maps.append(m)

    trace = os.environ.get("KERNEL_TRACE", "0") == "1"
    kw = {}
    if trace:
        kw = dict(trace=True, tmpdir=os.environ.get("KERNEL_TRACE_DIR") or None)
    res = run_bass_kernel_spmd(nc, in_maps, core_ids=list(range(N_CORES)), **kw)
    global LAST_RESULT
    LAST_RESULT = res
    outs = [res.results[c]["out"].reshape(B_LOCAL, D) for c in range(N_CORES)]
    return np.concatenate(outs, axis=0).astype(np.float32)


if __name__ == "__main__":
    pass
